# revision 34
# baseline (speedup 1.0000x reference)
"""Trainium2 Bass kernel for a 2-layer spiking NN (snntorch Leaky, reset='subtract').

Reference semantics (per time step t, fp32):
    cur1 = x_t @ w1.T + b1
    mem1 = beta*mem1 + cur1 - spk1          ; spk1 = (mem1 - 1 > 0)
    cur2 = spk1 @ w2.T + b2
    mem2 = beta*mem2 + cur2 - spk2          ; spk2 = (mem2 - 1 > 0)
    out  = sum_t spk2                        # [B, OUT] spike counts

Shapes: T=25, B=1024, IN=1024, HID=4096, OUT=64.  Data-parallel over 8
NeuronCores: each core runs 128 batch rows; outputs are concatenated.

Layer-1 matmuls run entirely in fp8 DoubleRow mode (1.0 cyc/row vs 1.25 for
the fp16-hi + fp8-lo split): the doubled weight V = 2*w1 is split into FOUR
e4m3 terms at per-term scales (2^10, 2^14, 2^16, 2^16) -- each cascade stage
stays in e4m3's normal range so the residual after 4 terms is ~2^-17
relative, BETTER than an fp16+e4m3 split.  The moving operand is binary x
encoded at the three matching power-of-2 scales in e5m2 (exact).  DR-mode
PSUM accumulation adds ~2e-6-relative noise (measured on HW); a CPU
flip-study with the empirical noise distribution predicted 1-3 flipped spike
counts, and hardware measures 2 flips (rel err 1.27e-2, inside the 2e-2
gate; the old baseline measured 3 flips / 1.55e-2).

The membrane recurrence consumes sigma = Sign(mem - th) in {-1,0,1} directly
(spk = (sigma+1)/2 folded into doubled state + bias constants), which removes
the Pool max() stage entirely: per chunk the chain is
    DVE: q = (v + k_c) + Z (psum)   ACT: sigma = Sign(q) [fp8e5, multi-chunk]
    DVE: v = beta*q - sigma
Layer-2 also runs in DoubleRow fp8: stationary = sigma chunk-pairs (exact
+-1 in e5m2), moving = four e4m3 terms of w2.T/2 at scale 2^12 (2048 cyc/t
vs 4096 for the packed-fp16 scheme; flip-study: zero added flips).
"""

import os
import sys

for _p in ("/opt/trn_rl_repo", "/opt/pypackages"):
    if os.path.isdir(_p) and _p not in sys.path:
        sys.path.insert(0, _p)

import numpy as np
import ml_dtypes

import concourse.bacc as bacc
import concourse.mybir as mybir
import concourse.tile as tile
from concourse.bass_utils import run_bass_kernel_spmd

T, B, IN, HID, OUT = 25, 1024, 1024, 4096, 64
NCORES = 8
BC = B // NCORES          # 128 batch rows per core
JC = IN // 256            # 4 contraction chunk-pairs (DoubleRow)
NCH = HID // 128          # 32 neuron chunks
L2P = NCH // 2            # 16 layer-2 contraction chunk-pairs
BETA = 0.9
NT1 = 4                   # layer-1 e4m3 terms
# per-term scales for V = 2*w1 (|V|max ~0.21 -> 2^10*0.21=215<240); x encodings
# are x*2^-s, exact in e5m2 down to the 2^-16 subnormal.
SC1 = [2.0 ** 10, 2.0 ** 14, 2.0 ** 16, 2.0 ** 16]
XSC = [2.0 ** -10, 2.0 ** -14, 2.0 ** -16]   # term 2,3 share the last
XIDX = [0, 1, 2, 2]       # term -> x-encoding index
NXE = 3
NT2 = 4                   # layer-2 e4m3 terms, single scale
SC2 = 2.0 ** 12           # scale for w2.T/2 (|w2/2|max*2^12 ~ 192 < 240)

F32 = mybir.dt.float32
F16 = mybir.dt.float16
F8E4 = mybir.dt.float8e4
F8E5 = mybir.dt.float8e5
DRMODE = mybir.MatmulPerfMode.DoubleRow
E4NP = ml_dtypes.float8_e4m3
E5NP = ml_dtypes.float8_e5m2
ADD = mybir.AluOpType.add
MULT = mybir.AluOpType.mult
SUB = mybir.AluOpType.subtract
ISGT = mybir.AluOpType.is_gt
SIGN = mybir.ActivationFunctionType.Sign

# time groups: quad first (absorbs the 16MB weight-DMA ramp at a lower PE
# consumption rate), then a triple, 8 pairs, and 2 singles (the singles keep
# the final recurrence drain short once L1 matmuls stop hiding its latency)
GROUPS = ([(0, 4), (4, 3)] + [(7 + 2 * i, 2) for i in range(8)]
          + [(23, 1), (24, 1)])
XCOLS = 8 * 128 * T       # flat x dram cols per encoding (j-major per group)

_CACHE: dict = {}


def _build_nc():
    nc = bacc.Bacc("TRN2", target_bir_lowering=False, debug=False)

    w1t_d = [nc.dram_tensor(f"w1t{k}", [128, NCH * 1024], F8E4,
                            kind="ExternalInput") for k in range(NT1)]
    x8_d = [nc.dram_tensor(f"x8{e}", [128, XCOLS], F8E5, kind="ExternalInput")
            for e in range(NXE)]
    # smalls packed: [k1 | v0 | th2 | s20] = 32+32+64+64 f32 cols
    sml_d = nc.dram_tensor("sml", [128, 2 * NCH + 2 * OUT], F32,
                           kind="ExternalInput")
    w2t_d = nc.dram_tensor("w2t", [128, NT2 * L2P * 2 * OUT], F8E4,
                           kind="ExternalInput")
    out_d = nc.dram_tensor("out", [128, OUT], F32, kind="ExternalOutput")

    with tile.TileContext(nc) as tc:
        with (
            tc.tile_pool(name="const", bufs=1) as cpool,
            tc.tile_pool(name="state", bufs=1) as spool,
            tc.tile_pool(name="xin", bufs=2) as xpool,
            tc.tile_pool(name="sg", bufs=20) as gpool,
            tc.tile_pool(name="stg", bufs=8) as stpool,
            tc.tile_pool(name="small", bufs=4) as mpool,
            tc.tile_pool(name="l1p", bufs=2, space="PSUM") as l1pool,
            tc.tile_pool(name="po2", bufs=4, space="PSUM") as o2pool,
        ):
            w1t = [cpool.tile([128, NCH, JC, 2, 128], F8E4, tag=f"w1t{k}",
                              name=f"w1t{k}") for k in range(NT1)]
            nth1 = cpool.tile([128, NCH], F32, tag="nth1")
            v0 = cpool.tile([128, NCH], F32, tag="v0")
            w2t = cpool.tile([128, NT2, L2P, 2, OUT], F8E4, tag="w2t")
            th2 = cpool.tile([128, OUT], F32, tag="th2")
            s1 = spool.tile([128, NCH, 128], F32, tag="s1")
            s2 = spool.tile([128, OUT], F32, tag="s2")
            cnt = spool.tile([128, OUT], F32, tag="cnt")

            # ---- init DMAs (one packed transfer for the small constants) ----
            sml = cpool.tile([128, 2 * NCH + 2 * OUT], F32, tag="sml")
            nc.sync.dma_start(sml[:], sml_d[:, :])
            nc.vector.tensor_copy(nth1[:], sml[:, :NCH])
            nc.vector.tensor_copy(v0[:], sml[:, NCH:2 * NCH])
            nc.vector.tensor_copy(th2[:], sml[:, 2 * NCH:2 * NCH + OUT])
            nc.vector.tensor_copy(s2[:], sml[:, 2 * NCH + OUT:])
            # w1 terms stream in 16 per-2-chunk pieces (4 tensors each),
            # emitted just-in-time in the wave loop on alternating queues
            def load_w_piece(q):
                cs = slice(q * 2048, (q + 1) * 2048)
                for k in range(NT1):
                    eng = nc.gpsimd if (k % 2 == 0) else nc.sync
                    eng.dma_start(w1t[k][:, q * 2:(q + 1) * 2, :, :, :],
                                  w1t_d[k][:, cs])

            load_w_piece(0)
            # s1 state init: s1[p, c, b] = v0[p, c]
            nc.vector.memset(s1[:], 0.0)
            nc.vector.memset(cnt[:], 0.0)
            for c in range(NCH):
                nc.vector.tensor_scalar(s1[:, c, :], s1[:, c, :],
                                        v0[:, c:c + 1], None, ADD)

            # ---- wave descriptors (flat across groups) ----
            waves = []
            for gi, (t0, glen) in enumerate(GROUPS):
                cpw = 2 if glen >= 3 else 4
                nw = NCH // cpw
                for w in range(nw):
                    waves.append(dict(
                        g=gi, glen=glen, cpw=cpw, c0=w * cpw,
                        first=(w == 0), last=(w == nw - 1)))
            NW = len(waves)

            xt8 = [None] * len(GROUPS)
            xoff = [0] * len(GROUPS)
            off = 0
            for gi, (t0, glen) in enumerate(GROUPS):
                xoff[gi] = off
                off += 8 * glen * 128

            def load_x(gi):
                glen = GROUPS[gi][1]
                tiles = []
                o = xoff[gi]
                cols = 8 * glen * 128
                for e in range(NXE):
                    xt = xpool.tile([128, JC, 2, glen * 128], F8E5,
                                    tag=f"x8e{e}", name=f"x8_{gi}_{e}")
                    tiles.append(xt)
                if gi == 0:
                    # j-split halves so the first matmuls start sooner
                    for half in range(2):
                        for e in range(NXE):
                            eng = nc.sync if ((e + half) % 2 == 0) else nc.gpsimd
                            js = slice(half * (JC // 2), (half + 1) * (JC // 2))
                            cs = slice(o + half * cols // 2,
                                       o + (half + 1) * cols // 2)
                            eng.dma_start(tiles[e][:, js, :, :], x8_d[e][:, cs])
                else:
                    for e in range(NXE):
                        eng = nc.sync if (e % 2 == 0) else nc.gpsimd
                        eng.dma_start(tiles[e][:], x8_d[e][:, o:o + cols])
                xt8[gi] = tiles

            load_x(0)

            # per-wave tile handles, filled during emission
            Pt = [None] * NW                       # l1 psum tile
            SG = [[None] * 4 for _ in range(NW)]   # sigma tiles per tl
            STG = [[None] * 4 for _ in range(NW)]  # staged psum copies, tl>=1
            o2t = {}                               # (gi, tl) -> l2 psum tile
            COPYF = mybir.ActivationFunctionType.Copy

            def emit_l1(i):
                wv = waves[i]
                gi, cpw, c0, glen = wv["g"], wv["cpw"], wv["c0"], wv["glen"]
                if wv["first"] and gi >= 1 and gi + 1 < len(GROUPS):
                    load_x(gi + 1)
                shape = [128, cpw, glen, 128]
                pad = shape if glen != 3 else [128, cpw, 4, 128]
                P = l1pool.tile(shape, F32, tag="P", name=f"P_{i}",
                                padded_shape=pad)
                Pt[i] = P
                xts = xt8[gi]
                for ci in range(cpw):
                    c = c0 + ci
                    pslice = P[:, ci, :, :]
                    nmm = NT1 * JC
                    n = 0
                    for k in range(NT1):
                        xt = xts[XIDX[k]]
                        for j in range(JC):
                            nc.tensor.matmul(pslice, w1t[k][:, c, j, :, :],
                                             xt[:, j, :, :], start=(n == 0),
                                             stop=(n == nmm - 1),
                                             perf_mode=DRMODE)
                            n += 1

            def emit_stage(i, tl):
                # ACT copies deep-tl psum slices to SBUF right after the wave
                # completes, so the psum tile recycles without waiting for the
                # DVE ladder to reach that timestep
                wv = waves[i]
                cpw = wv["cpw"]
                st = stpool.tile([128, cpw, 128], F32, tag="stg",
                                 name=f"stg_{i}_{tl}")
                STG[i][tl] = st
                nc.scalar.activation(st[:], Pt[i][:, :, tl, :], COPYF,
                                     bias=0.0, scale=1.0)

            def emit_op1(i, tl):
                # whole-wave add: s1 += Z (psum for tl 0, staged copy after)
                wv = waves[i]
                c0, cpw = wv["c0"], wv["cpw"]
                src = Pt[i][:, :, 0, :] if tl == 0 else STG[i][tl][:]
                sl = s1[:, c0:c0 + cpw, :]
                nc.vector.tensor_tensor(sl, sl, src, ADD)

            def emit_op1_final(i):
                # final timestep: state dies after sigma, so fold the
                # per-chunk threshold into op1 (shifted q) and let sigma run
                # bias-free over the whole wave in one ACT op
                wv = waves[i]
                c0, cpw = wv["c0"], wv["cpw"]
                P = Pt[i]
                for ci in range(cpw):
                    c = c0 + ci
                    sl = s1[:, c, :]
                    nc.vector.scalar_tensor_tensor(sl, sl, nth1[:, c:c + 1],
                                                   P[:, ci, 0, :], ADD, ADD)
                sg = gpool.tile([128, cpw, 128], F8E5, tag="sg",
                                name=f"sgf_{i}")
                SG[i][0] = sg
                nc.scalar.activation(sg[:], s1[:, c0:c0 + cpw, :], SIGN,
                                     bias=0.0, scale=1.0)

            def emit_sigma(i, tl):
                # per-chunk (the threshold bias is a [128,1] per-chunk column)
                wv = waves[i]
                c0, cpw = wv["c0"], wv["cpw"]
                sg = gpool.tile([128, cpw, 128], F8E5, tag="sg",
                                name=f"sg_{i}_{tl}")
                SG[i][tl] = sg
                for ci in range(cpw):
                    c = c0 + ci
                    nc.scalar.activation(sg[:, ci, :], s1[:, c, :], SIGN,
                                         bias=nth1[:, c:c + 1], scale=1.0)

            def emit_op3(i, tl):
                wv = waves[i]
                c0, cpw = wv["c0"], wv["cpw"]
                sl = s1[:, c0:c0 + cpw, :]
                nc.vector.scalar_tensor_tensor(sl, sl, BETA, SG[i][tl][:],
                                               MULT, SUB)

            def emit_l2(i, tl):
                wv = waves[i]
                gi, c0, cpw = wv["g"], wv["c0"], wv["cpw"]
                if (gi, tl) not in o2t:
                    o2t[(gi, tl)] = o2pool.tile([128, OUT], F32, tag="o2",
                                                name=f"o2_{gi}_{tl}")
                o2 = o2t[(gi, tl)][:]
                sg = SG[i][tl]
                np0 = c0 // 2
                npw = cpw // 2
                for pi in range(npw):
                    for k in range(NT2):
                        jj = np0 + pi
                        nc.tensor.matmul(
                            o2[:], sg[:, 2 * pi:2 * pi + 2, :],
                            w2t[:, k, jj, :, :],
                            start=(jj == 0 and k == 0),
                            stop=(jj == L2P - 1 and k == NT2 - 1),
                            perf_mode=DRMODE)

            def emit_l2rec(gi, tl):
                # returns closures so the caller can spread DVE work
                o2 = o2t.pop((gi, tl))[:]
                q2 = mpool.tile([128, OUT], F32, tag="q2", name=f"q2_{gi}_{tl}")
                spk2 = mpool.tile([128, OUT], F32, tag="spk2",
                                  name=f"spk2_{gi}_{tl}")
                inv = 1.0 / SC2
                return [
                    lambda: nc.vector.scalar_tensor_tensor(q2[:], o2, inv, s2[:], MULT, ADD),
                    lambda: nc.vector.tensor_tensor(spk2[:], q2[:], th2[:], ISGT),
                    lambda: nc.vector.scalar_tensor_tensor(s2[:], q2[:], BETA, spk2[:], MULT, SUB),
                    lambda: nc.vector.tensor_tensor(cnt[:], cnt[:], spk2[:], ADD),
                ]

            def has_tl(j, d):
                return 0 <= j < NW and d < waves[j]["glen"]

            # ---- software-pipelined emission ----
            l2_emitted = set()
            l2rec_ops = []
            for i in range(NW + 10):
                budget = 3 if i < NW - 2 else 16
                while l2rec_ops and budget > 0:
                    l2rec_ops.pop(0)()
                    budget -= 1
                if 0 <= i <= 14:
                    load_w_piece(i + 1)
                if i == 0:
                    nc.sync.dma_start(w2t[:], w2t_d[:, :])
                if i == 12:
                    load_x(1)
                if i < NW:
                    emit_l1(i)
                # DVE ladder: op3 one slot after sigma, op1 chained after op3.
                # The final group's op3 writes state nobody reads: skip it.
                for d in range(1, 5):
                    j = i - d
                    if has_tl(j, d - 1) and not (
                            waves[j]["g"] == len(GROUPS) - 1
                            and d - 1 == waves[j]["glen"] - 1):
                        emit_op3(j, d - 1)
                    if has_tl(j, d):
                        emit_op1(j, d)
                final = i < NW and waves[i]["g"] == len(GROUPS) - 1
                if i < NW and not final:
                    emit_op1(i, 0)
                if final:
                    emit_op1_final(i)
                # ACT sigmas for the op1s emitted this slot (t0 last)
                for d in range(1, 5):
                    if has_tl(i - d, d):
                        emit_sigma(i - d, d)
                if i < NW and not final:
                    emit_sigma(i, 0)
                # stage deep-tl psum slices after the sigmas (ACT order):
                # they only need to finish before the next-next wave
                if i < NW:
                    for tl in range(1, waves[i]["glen"]):
                        emit_stage(i, tl)
                # layer-2 matmuls lag behind sigma production; the layer-2
                # recurrence follows a group's last chunk
                for j in range(max(0, i - 8), i + 1):
                    if not (0 <= j < NW):
                        continue
                    lag = 1 if j >= NW - 16 else 2
                    for tl in range(waves[j]["glen"]):
                        if (j, tl) in l2_emitted or i < j + tl + lag:
                            continue
                        l2_emitted.add((j, tl))
                        emit_l2(j, tl)
                        if waves[j]["last"]:
                            l2rec_ops.extend(emit_l2rec(waves[j]["g"], tl))

            nc.sync.dma_start(out_d[:, :], cnt[:])

    nc.compile()
    return nc


def _cascade(w, scales):
    """Greedy e4m3 cascade of w at the given per-term scales.
    Returns list of e4m3 term arrays (stored at their scale)."""
    r = w.astype(np.float32).copy()
    terms = []
    for s in scales:
        t = np.clip(r * np.float32(s), -240.0, 240.0).astype(E4NP)
        terms.append(t)
        r = r - t.astype(np.float32) / np.float32(s)
    return terms


def _prep_shared(w1, b1, w2, b2):
    w1 = w1.astype(np.float32)
    V = (2.0 * w1).astype(np.float32)                 # doubled weights
    terms1 = _cascade(V, SC1)

    # term layout: [p, c*1024 + j*256 + s*128 + n] = t[c*128+n, j*256+s*128+p]
    def lay_lo(a):
        return np.ascontiguousarray(
            a.reshape(NCH, 128, JC, 2, 128).transpose(4, 0, 2, 3, 1)
            .reshape(128, NCH * 1024))

    w1t = {f"w1t{k}": lay_lo(terms1[k]) for k in range(NT1)}

    # layer 2: terms of Wh = w2.T/2 at scale SC2; moving layout
    # [p, k, pair, s, o] = term[h = pair*256 + s*128 + p, o]
    Wh = np.ascontiguousarray(w2.T.astype(np.float64) / 2.0)   # [HID, OUT]
    terms2 = _cascade(Wh.astype(np.float32), [SC2] * NT2)
    w2c = np.stack([t.reshape(L2P, 2, 128, OUT).transpose(2, 0, 1, 3)
                    for t in terms2], axis=1)        # [128, NT2, L2P, 2, OUT]
    w2c = np.ascontiguousarray(w2c.reshape(128, NT2 * L2P * 2 * OUT))

    # effective quantized Wh for the bias fold (exact in fp64)
    Whq = sum(t.astype(np.float64) for t in terms2) / SC2
    B2 = b2.astype(np.float64) + Whq.sum(axis=0)     # b2 + colsum(Wh)

    # layer-1 constants (doubled state, sigma recurrence):
    #   c = (2 b1 - 1)/(1-beta); th = 2 - c
    #   sigma = Sgn(q + (c - 2)); init state v0 = 1 - beta*c
    c1 = (2.0 * b1.astype(np.float64) - 1.0) / (1.0 - BETA)
    nn = c1 - 2.0
    vv = 1.0 - BETA * c1
    k1 = np.ascontiguousarray(nn.reshape(NCH, 128).T).astype(np.float32)
    v0 = np.ascontiguousarray(vv.reshape(NCH, 128).T).astype(np.float32)

    # layer-2 constants (plain spk recurrence, bias folded)
    c2 = (B2 / (1.0 - BETA))
    th2 = np.broadcast_to((1.0 - c2).astype(np.float32), (128, OUT))
    s20 = np.broadcast_to((-BETA * c2).astype(np.float32), (128, OUT))
    sml = np.ascontiguousarray(
        np.concatenate([k1, v0, th2, s20], axis=1).astype(np.float32))
    return {**w1t, "w2t": w2c, "sml": sml}


def _prep_x(spike_seq, core):
    xs = np.asarray(spike_seq[:, core * BC:(core + 1) * BC, :], dtype=np.float32)
    blocks = [[] for _ in range(NXE)]
    for t0, glen in GROUPS:
        blk = xs[t0:t0 + glen]                       # [glen, 128b, 1024in]
        b8 = blk.reshape(glen, 128, JC, 2, 128).transpose(4, 2, 3, 0, 1)
        flat = b8.reshape(128, JC * 2 * glen * 128)
        for e in range(NXE):
            blocks[e].append(flat * np.float32(XSC[e]))
    return [np.ascontiguousarray(np.concatenate(b, axis=1)).astype(E5NP)
            for b in blocks]


def kernel(spike_seq, w1, b1, w2, b2):
    if "nc" not in _CACHE:
        _CACHE["nc"] = _build_nc()
    nc = _CACHE["nc"]

    shared = _prep_shared(np.asarray(w1), np.asarray(b1), np.asarray(w2),
                          np.asarray(b2))
    in_maps = []
    for c in range(NCORES):
        xe = _prep_x(spike_seq, c)
        m = {f"x8{e}": xe[e] for e in range(NXE)}
        in_maps.append({**m, **shared})
    res = run_bass_kernel_spmd(nc, in_maps, core_ids=list(range(NCORES)))
    out = np.concatenate([res.results[c]["out"] for c in range(NCORES)], axis=0)
    return out.astype(np.asarray(spike_seq).dtype)


# revision 35
# speedup vs baseline: 1.0076x; 1.0076x over previous
"""Trainium2 Bass kernel for a 2-layer spiking NN (snntorch Leaky, reset='subtract').

Reference semantics (per time step t, fp32):
    cur1 = x_t @ w1.T + b1
    mem1 = beta*mem1 + cur1 - spk1          ; spk1 = (mem1 - 1 > 0)
    cur2 = spk1 @ w2.T + b2
    mem2 = beta*mem2 + cur2 - spk2          ; spk2 = (mem2 - 1 > 0)
    out  = sum_t spk2                        # [B, OUT] spike counts

Shapes: T=25, B=1024, IN=1024, HID=4096, OUT=64.  Data-parallel over 8
NeuronCores: each core runs 128 batch rows; outputs are concatenated.

Layer-1 matmuls run entirely in fp8 DoubleRow mode (1.0 cyc/row vs 1.25 for
the fp16-hi + fp8-lo split): the doubled weight V = 2*w1 is split into FOUR
e4m3 terms at per-term scales (2^10, 2^14, 2^16, 2^16) -- each cascade stage
stays in e4m3's normal range so the residual after 4 terms is ~2^-17
relative, BETTER than an fp16+e4m3 split.  The moving operand is binary x
encoded at the three matching power-of-2 scales in e5m2 (exact).  DR-mode
PSUM accumulation adds ~2e-6-relative noise (measured on HW); a CPU
flip-study with the empirical noise distribution predicted 1-3 flipped spike
counts, and hardware measures 2 flips (rel err 1.27e-2, inside the 2e-2
gate; the old baseline measured 3 flips / 1.55e-2).

The membrane recurrence consumes sigma = Sign(mem - th) in {-1,0,1} directly
(spk = (sigma+1)/2 folded into doubled state + bias constants), which removes
the Pool max() stage entirely: per chunk the chain is
    DVE: q = (v + k_c) + Z (psum)   ACT: sigma = Sign(q) [fp8e5, multi-chunk]
    DVE: v = beta*q - sigma
Layer-2 also runs in DoubleRow fp8: stationary = sigma chunk-pairs (exact
+-1 in e5m2), moving = four e4m3 terms of w2.T/2 at scale 2^12 (2048 cyc/t
vs 4096 for the packed-fp16 scheme; flip-study: zero added flips).
"""

import os
import sys

for _p in ("/opt/trn_rl_repo", "/opt/pypackages"):
    if os.path.isdir(_p) and _p not in sys.path:
        sys.path.insert(0, _p)

import numpy as np
import ml_dtypes

import concourse.bacc as bacc
import concourse.mybir as mybir
import concourse.tile as tile
from concourse.bass_utils import run_bass_kernel_spmd

T, B, IN, HID, OUT = 25, 1024, 1024, 4096, 64
NCORES = 8
BC = B // NCORES          # 128 batch rows per core
JC = IN // 256            # 4 contraction chunk-pairs (DoubleRow)
NCH = HID // 128          # 32 neuron chunks
L2P = NCH // 2            # 16 layer-2 contraction chunk-pairs
BETA = 0.9
NT1 = 4                   # layer-1 e4m3 terms
# per-term scales for V = 2*w1 (|V|max ~0.21 -> 2^10*0.21=215<240); x encodings
# are x*2^-s, exact in e5m2 down to the 2^-16 subnormal.
SC1 = [2.0 ** 10, 2.0 ** 14, 2.0 ** 16, 2.0 ** 16]
XSC = [2.0 ** -10, 2.0 ** -14, 2.0 ** -16]   # term 2,3 share the last
XIDX = [0, 1, 2, 2]       # term -> x-encoding index
NXE = 3
NT2 = 4                   # layer-2 e4m3 terms, single scale
SC2 = 2.0 ** 12           # scale for w2.T/2 (|w2/2|max*2^12 ~ 192 < 240)

F32 = mybir.dt.float32
F16 = mybir.dt.float16
F8E4 = mybir.dt.float8e4
F8E5 = mybir.dt.float8e5
DRMODE = mybir.MatmulPerfMode.DoubleRow
E4NP = ml_dtypes.float8_e4m3
E5NP = ml_dtypes.float8_e5m2
ADD = mybir.AluOpType.add
MULT = mybir.AluOpType.mult
SUB = mybir.AluOpType.subtract
ISGT = mybir.AluOpType.is_gt
SIGN = mybir.ActivationFunctionType.Sign

# time groups: quad first (absorbs the 16MB weight-DMA ramp at a lower PE
# consumption rate), then a triple, 8 pairs, and 2 singles (the singles keep
# the final recurrence drain short once L1 matmuls stop hiding its latency)
GROUPS = ([(0, 4), (4, 3)] + [(7 + 2 * i, 2) for i in range(8)]
          + [(23, 1), (24, 1)])
XCOLS = 8 * 128 * T       # flat x dram cols per encoding (j-major per group)

_CACHE: dict = {}


def _build_nc():
    nc = bacc.Bacc("TRN2", target_bir_lowering=False, debug=False)

    w1t_d = [nc.dram_tensor(f"w1t{k}", [128, NCH * 1024], F8E4,
                            kind="ExternalInput") for k in range(NT1)]
    x8_d = [nc.dram_tensor(f"x8{e}", [128, XCOLS], F8E5, kind="ExternalInput")
            for e in range(NXE)]
    # smalls packed: [k1 | v0 | th2 | s20] = 32+32+64+64 f32 cols
    sml_d = nc.dram_tensor("sml", [128, 2 * NCH + 2 * OUT], F32,
                           kind="ExternalInput")
    w2t_d = nc.dram_tensor("w2t", [128, NT2 * L2P * 2 * OUT], F8E4,
                           kind="ExternalInput")
    out_d = nc.dram_tensor("out", [128, OUT], F32, kind="ExternalOutput")

    with tile.TileContext(nc) as tc:
        with (
            tc.tile_pool(name="const", bufs=1) as cpool,
            tc.tile_pool(name="state", bufs=1) as spool,
            tc.tile_pool(name="xin", bufs=2) as xpool,
            tc.tile_pool(name="sg", bufs=20) as gpool,
            tc.tile_pool(name="stg", bufs=8) as stpool,
            tc.tile_pool(name="small", bufs=4) as mpool,
            tc.tile_pool(name="l1p", bufs=2, space="PSUM") as l1pool,
            tc.tile_pool(name="po2", bufs=4, space="PSUM") as o2pool,
        ):
            w1t = [cpool.tile([128, NCH, JC, 2, 128], F8E4, tag=f"w1t{k}",
                              name=f"w1t{k}") for k in range(NT1)]
            nth1 = cpool.tile([128, NCH], F32, tag="nth1")
            v0 = cpool.tile([128, NCH], F32, tag="v0")
            w2t = cpool.tile([128, NT2, L2P, 2, OUT], F8E4, tag="w2t")
            th2 = cpool.tile([128, OUT], F32, tag="th2")
            s1 = spool.tile([128, NCH, 128], F32, tag="s1")
            s2 = spool.tile([128, OUT], F32, tag="s2")
            cnt = spool.tile([128, OUT], F32, tag="cnt")

            # ---- init DMAs (one packed transfer for the small constants) ----
            sml = cpool.tile([128, 2 * NCH + 2 * OUT], F32, tag="sml")
            nc.sync.dma_start(sml[:], sml_d[:, :])
            nc.vector.tensor_copy(nth1[:], sml[:, :NCH])
            nc.vector.tensor_copy(v0[:], sml[:, NCH:2 * NCH])
            nc.vector.tensor_copy(th2[:], sml[:, 2 * NCH:2 * NCH + OUT])
            nc.vector.tensor_copy(s2[:], sml[:, 2 * NCH + OUT:])
            # w1 terms stream in 16 per-2-chunk pieces (4 tensors each),
            # emitted just-in-time in the wave loop on alternating queues
            def load_w_piece(q):
                cs = slice(q * 2048, (q + 1) * 2048)
                for k in range(NT1):
                    eng = nc.gpsimd if (k % 2 == 0) else nc.sync
                    eng.dma_start(w1t[k][:, q * 2:(q + 1) * 2, :, :, :],
                                  w1t_d[k][:, cs])

            load_w_piece(0)
            # s1 state init: s1[p, c, b] = v0[p, c]
            nc.vector.memset(s1[:], 0.0)
            nc.vector.memset(cnt[:], 0.0)
            for c in range(NCH):
                nc.vector.tensor_scalar(s1[:, c, :], s1[:, c, :],
                                        v0[:, c:c + 1], None, ADD)

            # ---- wave descriptors (flat across groups) ----
            waves = []
            for gi, (t0, glen) in enumerate(GROUPS):
                cpw = 2 if glen >= 3 else 4
                nw = NCH // cpw
                for w in range(nw):
                    waves.append(dict(
                        g=gi, glen=glen, cpw=cpw, c0=w * cpw,
                        first=(w == 0), last=(w == nw - 1)))
            NW = len(waves)

            xt8 = [None] * len(GROUPS)
            xoff = [0] * len(GROUPS)
            off = 0
            for gi, (t0, glen) in enumerate(GROUPS):
                xoff[gi] = off
                off += 8 * glen * 128

            def load_x(gi):
                glen = GROUPS[gi][1]
                tiles = []
                o = xoff[gi]
                cols = 8 * glen * 128
                for e in range(NXE):
                    xt = xpool.tile([128, JC, 2, glen * 128], F8E5,
                                    tag=f"x8e{e}", name=f"x8_{gi}_{e}")
                    tiles.append(xt)
                if gi == 0:
                    # j-split halves so the first matmuls start sooner
                    for half in range(2):
                        for e in range(NXE):
                            eng = nc.sync if ((e + half) % 2 == 0) else nc.gpsimd
                            js = slice(half * (JC // 2), (half + 1) * (JC // 2))
                            cs = slice(o + half * cols // 2,
                                       o + (half + 1) * cols // 2)
                            eng.dma_start(tiles[e][:, js, :, :], x8_d[e][:, cs])
                else:
                    for e in range(NXE):
                        eng = nc.sync if (e % 2 == 0) else nc.gpsimd
                        eng.dma_start(tiles[e][:], x8_d[e][:, o:o + cols])
                xt8[gi] = tiles

            load_x(0)

            # per-wave tile handles, filled during emission
            Pt = [None] * NW                       # l1 psum tile
            SG = [[None] * 4 for _ in range(NW)]   # sigma tiles per tl
            STG = [[None] * 4 for _ in range(NW)]  # staged psum copies, tl>=1
            o2t = {}                               # (gi, tl) -> l2 psum tile
            COPYF = mybir.ActivationFunctionType.Copy

            def emit_l1(i):
                wv = waves[i]
                gi, cpw, c0, glen = wv["g"], wv["cpw"], wv["c0"], wv["glen"]
                if wv["first"] and gi >= 1 and gi + 1 < len(GROUPS):
                    load_x(gi + 1)
                shape = [128, cpw, glen, 128]
                pad = shape if glen != 3 else [128, cpw, 4, 128]
                P = l1pool.tile(shape, F32, tag="P", name=f"P_{i}",
                                padded_shape=pad)
                Pt[i] = P
                xts = xt8[gi]
                for ci in range(cpw):
                    c = c0 + ci
                    pslice = P[:, ci, :, :]
                    nmm = NT1 * JC
                    n = 0
                    for k in range(NT1):
                        xt = xts[XIDX[k]]
                        for j in range(JC):
                            nc.tensor.matmul(pslice, w1t[k][:, c, j, :, :],
                                             xt[:, j, :, :], start=(n == 0),
                                             stop=(n == nmm - 1),
                                             perf_mode=DRMODE)
                            n += 1

            def emit_stage(i, tl):
                # ACT copies deep-tl psum slices to SBUF right after the wave
                # completes, so the psum tile recycles without waiting for the
                # DVE ladder to reach that timestep
                wv = waves[i]
                cpw = wv["cpw"]
                st = stpool.tile([128, cpw, 128], F32, tag="stg",
                                 name=f"stg_{i}_{tl}")
                STG[i][tl] = st
                nc.scalar.activation(st[:], Pt[i][:, :, tl, :], COPYF,
                                     bias=0.0, scale=1.0)

            def emit_op1(i, tl):
                # whole-wave add: s1 += Z (psum for tl 0, staged copy after)
                wv = waves[i]
                c0, cpw = wv["c0"], wv["cpw"]
                src = Pt[i][:, :, 0, :] if tl == 0 else STG[i][tl][:]
                sl = s1[:, c0:c0 + cpw, :]
                nc.vector.tensor_tensor(sl, sl, src, ADD)

            def emit_sigma(i, tl):
                # per-chunk (the threshold bias is a [128,1] per-chunk column)
                wv = waves[i]
                c0, cpw = wv["c0"], wv["cpw"]
                sg = gpool.tile([128, cpw, 128], F8E5, tag="sg",
                                name=f"sg_{i}_{tl}")
                SG[i][tl] = sg
                for ci in range(cpw):
                    c = c0 + ci
                    nc.scalar.activation(sg[:, ci, :], s1[:, c, :], SIGN,
                                         bias=nth1[:, c:c + 1], scale=1.0)

            def emit_op3(i, tl):
                wv = waves[i]
                c0, cpw = wv["c0"], wv["cpw"]
                sl = s1[:, c0:c0 + cpw, :]
                nc.vector.scalar_tensor_tensor(sl, sl, BETA, SG[i][tl][:],
                                               MULT, SUB)

            def emit_l2(i, tl):
                wv = waves[i]
                gi, c0, cpw = wv["g"], wv["c0"], wv["cpw"]
                if (gi, tl) not in o2t:
                    o2t[(gi, tl)] = o2pool.tile([128, OUT], F32, tag="o2",
                                                name=f"o2_{gi}_{tl}")
                o2 = o2t[(gi, tl)][:]
                sg = SG[i][tl]
                np0 = c0 // 2
                npw = cpw // 2
                for pi in range(npw):
                    for k in range(NT2):
                        jj = np0 + pi
                        nc.tensor.matmul(
                            o2[:], sg[:, 2 * pi:2 * pi + 2, :],
                            w2t[:, k, jj, :, :],
                            start=(jj == 0 and k == 0),
                            stop=(jj == L2P - 1 and k == NT2 - 1),
                            perf_mode=DRMODE)

            def emit_l2rec(gi, tl):
                # returns closures so the caller can spread DVE work
                o2 = o2t.pop((gi, tl))[:]
                q2 = mpool.tile([128, OUT], F32, tag="q2", name=f"q2_{gi}_{tl}")
                spk2 = mpool.tile([128, OUT], F32, tag="spk2",
                                  name=f"spk2_{gi}_{tl}")
                inv = 1.0 / SC2
                return [
                    lambda: nc.vector.scalar_tensor_tensor(q2[:], o2, inv, s2[:], MULT, ADD),
                    lambda: nc.vector.tensor_tensor(spk2[:], q2[:], th2[:], ISGT),
                    lambda: nc.vector.scalar_tensor_tensor(s2[:], q2[:], BETA, spk2[:], MULT, SUB),
                    lambda: nc.vector.tensor_tensor(cnt[:], cnt[:], spk2[:], ADD),
                ]

            def has_tl(j, d):
                return 0 <= j < NW and d < waves[j]["glen"]

            # ---- software-pipelined emission ----
            l2_emitted = set()
            l2rec_ops = []
            for i in range(NW + 10):
                budget = 3 if i < NW - 2 else 16
                while l2rec_ops and budget > 0:
                    l2rec_ops.pop(0)()
                    budget -= 1
                if 0 <= i <= 14:
                    load_w_piece(i + 1)
                if i == 0:
                    nc.sync.dma_start(w2t[:], w2t_d[:, :])
                if i == 12:
                    load_x(1)
                if i < NW:
                    emit_l1(i)
                # DVE ladder: op3 one slot after sigma, op1 chained after op3.
                # The final group's op3 writes state nobody reads: skip it.
                for d in range(1, 5):
                    j = i - d
                    if has_tl(j, d - 1) and not (
                            waves[j]["g"] == len(GROUPS) - 1
                            and d - 1 == waves[j]["glen"] - 1):
                        emit_op3(j, d - 1)
                    if has_tl(j, d):
                        emit_op1(j, d)
                if i < NW:
                    emit_op1(i, 0)
                # ACT sigmas for the op1s emitted this slot (t0 last)
                for d in range(1, 5):
                    if has_tl(i - d, d):
                        emit_sigma(i - d, d)
                if i < NW:
                    emit_sigma(i, 0)
                # stage deep-tl psum slices after the sigmas (ACT order):
                # they only need to finish before the next-next wave
                if i < NW:
                    for tl in range(1, waves[i]["glen"]):
                        emit_stage(i, tl)
                # layer-2 matmuls lag behind sigma production; the layer-2
                # recurrence follows a group's last chunk
                for j in range(max(0, i - 8), i + 1):
                    if not (0 <= j < NW):
                        continue
                    lag = 1 if j >= NW - 16 else 2
                    for tl in range(waves[j]["glen"]):
                        if (j, tl) in l2_emitted or i < j + tl + lag:
                            continue
                        l2_emitted.add((j, tl))
                        emit_l2(j, tl)
                        if waves[j]["last"]:
                            l2rec_ops.extend(emit_l2rec(waves[j]["g"], tl))

            nc.sync.dma_start(out_d[:, :], cnt[:])

    nc.compile()
    return nc


def _cascade(w, scales):
    """Greedy e4m3 cascade of w at the given per-term scales.
    Returns list of e4m3 term arrays (stored at their scale)."""
    r = w.astype(np.float32).copy()
    terms = []
    for s in scales:
        t = np.clip(r * np.float32(s), -240.0, 240.0).astype(E4NP)
        terms.append(t)
        r = r - t.astype(np.float32) / np.float32(s)
    return terms


def _prep_shared(w1, b1, w2, b2):
    w1 = w1.astype(np.float32)
    V = (2.0 * w1).astype(np.float32)                 # doubled weights
    terms1 = _cascade(V, SC1)

    # term layout: [p, c*1024 + j*256 + s*128 + n] = t[c*128+n, j*256+s*128+p]
    def lay_lo(a):
        return np.ascontiguousarray(
            a.reshape(NCH, 128, JC, 2, 128).transpose(4, 0, 2, 3, 1)
            .reshape(128, NCH * 1024))

    w1t = {f"w1t{k}": lay_lo(terms1[k]) for k in range(NT1)}

    # layer 2: terms of Wh = w2.T/2 at scale SC2; moving layout
    # [p, k, pair, s, o] = term[h = pair*256 + s*128 + p, o]
    Wh = np.ascontiguousarray(w2.T.astype(np.float64) / 2.0)   # [HID, OUT]
    terms2 = _cascade(Wh.astype(np.float32), [SC2] * NT2)
    w2c = np.stack([t.reshape(L2P, 2, 128, OUT).transpose(2, 0, 1, 3)
                    for t in terms2], axis=1)        # [128, NT2, L2P, 2, OUT]
    w2c = np.ascontiguousarray(w2c.reshape(128, NT2 * L2P * 2 * OUT))

    # effective quantized Wh for the bias fold (exact in fp64)
    Whq = sum(t.astype(np.float64) for t in terms2) / SC2
    B2 = b2.astype(np.float64) + Whq.sum(axis=0)     # b2 + colsum(Wh)

    # layer-1 constants (doubled state, sigma recurrence):
    #   c = (2 b1 - 1)/(1-beta); th = 2 - c
    #   sigma = Sgn(q + (c - 2)); init state v0 = 1 - beta*c
    c1 = (2.0 * b1.astype(np.float64) - 1.0) / (1.0 - BETA)
    nn = c1 - 2.0
    vv = 1.0 - BETA * c1
    k1 = np.ascontiguousarray(nn.reshape(NCH, 128).T).astype(np.float32)
    v0 = np.ascontiguousarray(vv.reshape(NCH, 128).T).astype(np.float32)

    # layer-2 constants (plain spk recurrence, bias folded)
    c2 = (B2 / (1.0 - BETA))
    th2 = np.broadcast_to((1.0 - c2).astype(np.float32), (128, OUT))
    s20 = np.broadcast_to((-BETA * c2).astype(np.float32), (128, OUT))
    sml = np.ascontiguousarray(
        np.concatenate([k1, v0, th2, s20], axis=1).astype(np.float32))
    return {**w1t, "w2t": w2c, "sml": sml}


def _prep_x(spike_seq, core):
    xs = np.asarray(spike_seq[:, core * BC:(core + 1) * BC, :], dtype=np.float32)
    blocks = [[] for _ in range(NXE)]
    for t0, glen in GROUPS:
        blk = xs[t0:t0 + glen]                       # [glen, 128b, 1024in]
        b8 = blk.reshape(glen, 128, JC, 2, 128).transpose(4, 2, 3, 0, 1)
        flat = b8.reshape(128, JC * 2 * glen * 128)
        for e in range(NXE):
            blocks[e].append(flat * np.float32(XSC[e]))
    return [np.ascontiguousarray(np.concatenate(b, axis=1)).astype(E5NP)
            for b in blocks]


def kernel(spike_seq, w1, b1, w2, b2):
    if "nc" not in _CACHE:
        _CACHE["nc"] = _build_nc()
    nc = _CACHE["nc"]

    shared = _prep_shared(np.asarray(w1), np.asarray(b1), np.asarray(w2),
                          np.asarray(b2))
    in_maps = []
    for c in range(NCORES):
        xe = _prep_x(spike_seq, c)
        m = {f"x8{e}": xe[e] for e in range(NXE)}
        in_maps.append({**m, **shared})
    res = run_bass_kernel_spmd(nc, in_maps, core_ids=list(range(NCORES)))
    out = np.concatenate([res.results[c]["out"] for c in range(NCORES)], axis=0)
    return out.astype(np.asarray(spike_seq).dtype)
